# revision 25
# baseline (speedup 1.0000x reference)
"""MoE layer (top-2 of 8 experts) on 8 TRN2 NeuronCores.

Strategy (paired-expert tensor-parallel):
  Host: gate logits (fp64) + softmax + top-2 + renormalized weights — the
      routing / sharding decision. Experts are paired heavy+light by token
      count; cores 2i and 2i+1 each process the PAIR's full token list but
      only half of the hidden dim H (column-split W1, row-split W2), so
      every core does ~(B*K/E) tokens of half-H work — near-perfect load
      balance. Partial outputs of the two halves are summed on the host.
  Device (one launch): per block of 256 tokens, L1 (x@W1-half) and
      L2 (h@W2-half) are interleaved per h-tile so weight streaming is
      spread across each block's whole span. All weight/x inputs are
      host-packed so every DMA chunk is contiguous per partition (4KB
      lines). bf16 matmuls, fp32 PSUM. The drain scales by the routing
      weight (free: it rides the PSUM->SBUF copy).
  Host: sum half-contributions, scatter-add per token, add b2 if nonzero.
"""

import numpy as np
import ml_dtypes

import concourse.mybir as mybir
import concourse.tile as tile
from concourse import bacc
from concourse.bass_utils import run_bass_kernel_spmd

P = 128
N_CORES = 8
CB = 256  # token block
LAG = 2   # L2 trails L1 by this many h-tiles
BF16 = mybir.dt.bfloat16
F32 = mybir.dt.float32
_bf16_np = ml_dtypes.bfloat16

_build_cache = {}


def _block_list(c0, c1, final_small=False):
    """(start, size) blocks; optionally end with two 128-blocks so the
    kernel's tail drain is half-sized."""
    out = []
    pos = c0
    while pos < c1:
        rem = c1 - pos
        if final_small and rem == CB:
            out += [(pos, P), (pos + P, P)]
            pos = c1
            break
        cb = CB if rem >= CB else P
        out.append((pos, cb))
        pos += cb
    return out


# W1 chunk schedule (h-tile units): lead with single-tile chunks so the
# first matmul starts after ~256KB of DMA.
def _w1_chunks(HO, small_head):
    if small_head:
        # single-tile chunks through the startup ramp so each early
        # LDWEIGHTS waits on only 256KB of delivery
        n1 = min(6, HO)
        return [(h, 1) for h in range(n1)] + [(h, 2) for h in range(n1, HO, 2)]
    return [(h, 2) for h in range(0, HO, 2)]


def _build_pair(D, H2, O, Ca, Cb):
    """One core's half-H FFN over two experts' (padded) routed tokens.

    Tokens [0, Ca) use expert-set 0's weights, [Ca, Ca+Cb) expert-set 1's.
    y[C, O] = relu(x @ W1half + b1half) @ W2half * w_token[:, None]
    (partial over H — the other core of the pair holds the other half).

    Packed input layouts (host-prepared, all contiguous per partition):
      xT:  [P, DO*C]    block-major [blk][do][cb]
      w1s: [P, DO*H2]   chunk-major [chunk][do][nh*P]
      w2s: [P, H2*O//P] chunk-major [chunk][hc][O]
    """
    C = Ca + Cb
    nc = bacc.Bacc(None, target_bir_lowering=False)
    DO, HO = D // P, H2 // P
    OO = O // 512
    HC = 2                   # h-tiles per W2 chunk
    NWC = HO // HC
    xT = nc.dram_tensor("xT", [P, DO * C], BF16, kind="ExternalInput")
    w1s = [nc.dram_tensor(f"w1{s}", [P, DO * H2], BF16, kind="ExternalInput") for s in range(2)]
    w2s = [nc.dram_tensor(f"w2{s}", [P, H2 * O // P], BF16, kind="ExternalInput") for s in range(2)]
    b1s = [nc.dram_tensor(f"b1{s}", [P, HO], F32, kind="ExternalInput") for s in range(2)]
    wt = nc.dram_tensor("wt", [P, C // P], F32, kind="ExternalInput")
    y = nc.dram_tensor("y", [C, O], F32, kind="ExternalOutput")
    blocks = [(n0, cb, 0) for n0, cb in _block_list(0, Ca)] + \
             [(n0, cb, 1) for n0, cb in _block_list(Ca, C)]
    nA = sum(1 for b in blocks if b[2] == 0)
    # startup super-block: blocks 0+1 share segment-0 weights; running both
    # blocks' L1s interleaved and splitting L2 into two O-phases halves the
    # startup weight-demand rate. Needs 2 leading full blocks in segment 0.
    sblk = nA >= 7 and blocks[0][1] == CB and blocks[1][1] == CB
    xoff = {}
    off = 0
    for n0, cb, _ in blocks:
        xoff[n0] = off
        off += DO * cb
    w1_chunks = [_w1_chunks(HO, True), _w1_chunks(HO, False)]
    w1_off = []
    w1_of_hi = []
    for s in range(2):
        offs, m, off = [], {}, 0
        for ci, (h0, nh) in enumerate(w1_chunks[s]):
            offs.append(off)
            off += DO * nh * P
            for j in range(nh):
                m[h0 + j] = (ci, j)
        w1_off.append(offs)
        w1_of_hi.append(m)
    y_r = y.rearrange("(n p) o -> p n o", p=P)
    with tile.TileContext(nc) as tc:
        with (
            tc.tile_pool(name="wpool", bufs=1) as wp,
            tc.tile_pool(name="xpool", bufs=4) as xp,
            tc.tile_pool(name="hpool", bufs=4) as hp,
            tc.tile_pool(name="hsbpool", bufs=1) as hsbp,
            tc.tile_pool(name="opool", bufs=4) as op,
            tc.tile_pool(name="hps", bufs=2, space="PSUM") as hps,
            tc.tile_pool(name="yps", bufs=1, space="PSUM") as yps,
        ):
            # --- startup DMAs ---
            # x of block 0 (and 1, for the super-block) + consts + paced W2
            # on scalar; segment-0 W1 chunks alternate sync/gpsimd.
            nx0 = 2 if sblk else 1
            x0t = []
            for b in range(nx0):
                t = xp.tile([P, DO * CB], BF16, tag="x", name=f"x{b}")
                cb0 = blocks[b][1]
                o0 = xoff[blocks[b][0]]
                if b == 0:
                    # two half-DMAs so the first L1's leading matmuls can
                    # start after ~half the x block has landed
                    hx = DO * cb0 // 2
                    nc.scalar.dma_start(t[:, :hx], xT[:, o0:o0 + hx])
                    nc.scalar.dma_start(t[:, hx:DO * cb0], xT[:, o0 + hx:o0 + DO * cb0])
                else:
                    nc.scalar.dma_start(t[:, :DO * cb0], xT[:, o0:o0 + DO * cb0])
                x0t.append(t)
            w1t = [[wp.tile([P, DO * nh * P], BF16, tag=f"w1_{s}_{k}", name=f"w1_{s}_{k}")
                    for k, (h0, nh) in enumerate(w1_chunks[s])] for s in range(2)]
            for k in range(len(w1_chunks[0])):
                eng = nc.sync if k % 2 == 0 else nc.gpsimd
                w, o = w1t[0][k], w1_off[0][k]
                if k == 0:
                    # split the startup-critical first chunk so its first
                    # d-tiles are LDWEIGHTS-able sooner
                    hw = w.shape[1] // 2
                    eng.dma_start(w[:, :hw], w1s[0][:, o:o + hw])
                    eng.dma_start(w[:, hw:], w1s[0][:, o + hw:o + w.shape[1]])
                else:
                    eng.dma_start(w[:], w1s[0][:, o:o + w.shape[1]])
            b1t = []
            for s in range(2):
                t = wp.tile([P, HO], F32, tag=f"b1_{s}")
                nc.scalar.dma_start(t[:], b1s[s][:])
                b1t.append(t)
            wt_sb = wp.tile([P, C // P], F32, tag="wt")
            nc.scalar.dma_start(wt_sb[:], wt[:])
            w2t = [[wp.tile([P, HC * O], BF16, tag=f"w2_{s}_{k}", name=f"w2_{s}_{k}")
                    for k in range(NWC)] for s in range(2)]
            nc.scalar.dma_start(w2t[0][0][:], w2s[0][:, 0:HC * O])

            # --- paced weight deliveries: {(key, hi): [(engine, tile, src)]} ---
            # keys: super-block L1s use keys 0 (block 0) and 1 (block 1);
            # regular blocks use their block index.
            paced = {}
            pace_w2a_key = 1 if sblk else 0
            for k in range(1, NWC):
                paced.setdefault((pace_w2a_key, max(0, 2 * k - (2 if sblk else 3))), []).append(
                    (nc.scalar, w2t[0][k], w2s[0][:, k * HC * O:(k + 1) * HC * O]))
            first_reg = 2 if sblk else 1
            if nA >= first_reg + 5:
                for k in range(len(w1_chunks[1])):
                    blk = first_reg + k // 2
                    paced.setdefault((blk, (k % 2) * (HO // 2)), []).append(
                        (nc.gpsimd, w1t[1][k],
                         w1s[1][:, w1_off[1][k]:w1_off[1][k] + w1t[1][k].shape[1]]))
                for k in range(NWC):
                    blk = min(nA - 1, first_reg + 4 + k // 3)
                    paced.setdefault((blk, (k % 3) * (HO // 3)), []).append(
                        (nc.scalar, w2t[1][k], w2s[1][:, k * HC * O:(k + 1) * HC * O]))
            else:
                for k in range(len(w1_chunks[1])):
                    nc.gpsimd.dma_start(
                        w1t[1][k][:],
                        w1s[1][:, w1_off[1][k]:w1_off[1][k] + w1t[1][k].shape[1]])
                for k in range(NWC):
                    nc.scalar.dma_start(w2t[1][k][:],
                                        w2s[1][:, k * HC * O:(k + 1) * HC * O])

            ndma = [0]
            last_bi = len(blocks) - 1

            def l1_tile(key, seg, hi, x_ap, cb, store, pool, tag):
                ph = hps.tile([P, CB], F32, tag="ph", name="ph")[:, :cb]
                ci, off = w1_of_hi[seg][hi]
                nh = w1_chunks[seg][ci][1]
                for di in range(DO):
                    nc.tensor.matmul(
                        ph[:],
                        w1t[seg][ci][:, (di * nh + off) * P:(di * nh + off + 1) * P],
                        x_ap[:, di * cb:(di + 1) * cb],
                        start=(di == 0),
                        stop=(di == DO - 1),
                    )
                ht = pool.tile([P, CB], BF16, tag=tag, name=tag)[:, :cb]
                act = nc.scalar.activation(
                    ht, ph[:],
                    mybir.ActivationFunctionType.Relu,
                    bias=b1t[seg][:, hi:hi + 1],
                )
                store[(key, hi)] = ht
                for eng, wtile, src in paced.pop((key, hi), []):
                    dma = eng.dma_start(wtile[:], src)
                    tile.add_dep_helper(
                        dma.ins, act.ins,
                        reason="pace weight stream behind compute",
                    )

            def drain(bank, n_idx, ot, flip, final=False):
                o_sb = op.tile([P, 512], F32, tag="o")
                if flip:
                    nc.scalar.activation(
                        o_sb[:], bank[:],
                        mybir.ActivationFunctionType.Copy,
                        scale=wt_sb[:, n_idx:n_idx + 1],
                    )
                else:
                    nc.vector.tensor_scalar_mul(
                        o_sb[:], bank[:], wt_sb[:, n_idx:n_idx + 1]
                    )
                # y normally on the sync (HWDGE) ring: gpsimd (SWDGE) DMAs
                # near the kernel end make the final queue DRAIN take ~6us,
                # and scalar-ring y DMAs block queued relus in the strict
                # FIFO. The FINAL block has no relus left to block, so its
                # DMAs split across both HWDGE rings to halve the tail.
                eng = nc.scalar if (final and not flip) else nc.sync
                eng.dma_start(y_r[:, n_idx, ot * 512:(ot + 1) * 512], o_sb[:])

            if sblk:
                # --- super-block over blocks 0+1 (segment 0) ---
                G = [(b, ct) for b in range(2) for ct in range(2)]
                hsb = {}
                for ot in range(OO):  # one O-phase at a time: 4 PSUM banks
                    ybp = {g: yps.tile([P, 512], F32, tag=f"y{g[0]}{g[1]}",
                                       name=f"ys{g[0]}{g[1]}") for g in G}

                    def l2_sb(hj, ot=ot, ybp=ybp):
                        for b, ct in G:
                            nc.tensor.matmul(
                                ybp[(b, ct)][:],
                                hsb[(b, hj)][:, ct * P:(ct + 1) * P],
                                w2t[0][hj // HC][:, (hj % HC) * O + ot * 512:
                                                 (hj % HC) * O + (ot + 1) * 512],
                                start=(hj == 0),
                                stop=(hj == HO - 1),
                            )

                    if ot == 0:
                        # block-1's L1s lag block-0's by 2 h-tiles so x1 is
                        # off the first-matmul critical path
                        for u in range(HO + 2):
                            if u < HO:
                                l1_tile(0, 0, u, x0t[0][:, :DO * CB], CB,
                                        hsb, hsbp, f"hsb0{u}")
                            if 2 <= u:
                                l1_tile(1, 0, u - 2, x0t[1][:, :DO * CB], CB,
                                        hsb, hsbp, f"hsb1{u - 2}")
                            if u >= 4:
                                l2_sb(u - 4)
                        for hj in range(HO - 2, HO):
                            l2_sb(hj)
                    else:
                        # g-major: each bank's accumulation starts as soon as
                        # its own phase-A drain completes
                        for b, ct in G:
                            for hj in range(HO):
                                nc.tensor.matmul(
                                    ybp[(b, ct)][:],
                                    hsb[(b, hj)][:, ct * P:(ct + 1) * P],
                                    w2t[0][hj // HC][:, (hj % HC) * O + ot * 512:
                                                     (hj % HC) * O + (ot + 1) * 512],
                                    start=(hj == 0),
                                    stop=(hj == HO - 1),
                                )
                    for gi, (b, ct) in enumerate(G):
                        drain(ybp[(b, ct)], b * 2 + ct, ot, gi % 2 == 0)

            for bi, (n0, cb, seg) in enumerate(blocks):
                if sblk and bi < 2:
                    continue
                if not sblk and bi == 0:
                    x_sb = x0t[0][:, :DO * cb]
                else:
                    x_sb = xp.tile([P, DO * CB], BF16, tag="x", name="x_sb")[:, :DO * cb]
                    nc.gpsimd.dma_start(x_sb[:], xT[:, xoff[n0]:xoff[n0] + DO * cb])
                nct = cb // P
                # the final (128-token) block gets its own PSUM tags so its
                # accumulation never waits on the previous block's drains
                tags = [f"y2{ot}" for ot in range(OO)] if (bi == last_bi and nct == 1) \
                    else None
                yb = [[yps.tile([P, 512], F32,
                                tag=(tags[ot] if tags else f"y{ct}{ot}"),
                                name=f"yb{ct}{ot}")
                       for ot in range(OO)] for ct in range(nct)]
                hts = {}

                def do_l2(hj, seg=seg, hts=hts, yb=yb, nct=nct, bi=bi):
                    ht = hts.pop((bi, hj))
                    for ct in range(nct):
                        for ot in range(OO):
                            nc.tensor.matmul(
                                yb[ct][ot][:],
                                ht[:, ct * P:(ct + 1) * P],
                                w2t[seg][hj // HC][:, (hj % HC) * O + ot * 512:
                                                   (hj % HC) * O + (ot + 1) * 512],
                                start=(hj == 0),
                                stop=(hj == HO - 1),
                            )

                for hi in range(HO):
                    l1_tile(bi, seg, hi, x_sb, cb, hts, hp, "h")
                    if hi >= LAG:
                        do_l2(hi - LAG)
                for hj in range(HO - LAG, HO):
                    do_l2(hj)

                for ct in range(nct):
                    n_idx = n0 // P + ct
                    for ot in range(OO):
                        drain(yb[ct][ot], n_idx, ot, (ct + ot) % 2 == 0,
                              final=(bi == last_bi))
    nc.finalize()
    return nc


def _pack_x(xT_pad, blocks, D, C):
    """[D, C] -> [P, DO*C] block-major [blk][do][cb], contiguous/partition."""
    DO = D // P
    out = np.empty((P, DO * C), dtype=xT_pad.dtype)
    off = 0
    for n0, cb in blocks:
        blkv = xT_pad[:, n0:n0 + cb].reshape(DO, P, cb).transpose(1, 0, 2)
        out[:, off:off + DO * cb] = blkv.reshape(P, DO * cb)
        off += DO * cb
    return np.ascontiguousarray(out)


def _pack_w1(w1h, chunks, D, H2):
    """[D, H2] -> [P, DO*H2] chunk-major [chunk][do][nh*P]."""
    DO = D // P
    out = np.empty((P, DO * H2), dtype=w1h.dtype)
    off = 0
    for h0, nh in chunks:
        c = w1h[:, h0 * P:(h0 + nh) * P].reshape(DO, P, nh * P).transpose(1, 0, 2)
        out[:, off:off + DO * nh * P] = c.reshape(P, DO * nh * P)
        off += DO * nh * P
    return np.ascontiguousarray(out)


def _pack_w2(w2h, H2, O, HC):
    """[H2, O] -> [P, H2*O//P] chunk-major [chunk][hc][O]."""
    HO = H2 // P
    c = w2h.reshape(HO // HC, HC, P, O).transpose(2, 0, 1, 3)
    return np.ascontiguousarray(c.reshape(P, H2 * O // P))


def kernel(x, W1, b1, W2, b2, gate_w, gate_b):
    x = np.ascontiguousarray(x, dtype=np.float32)
    W1 = np.asarray(W1, dtype=np.float32)
    b1 = np.asarray(b1, dtype=np.float32)
    W2 = np.asarray(W2, dtype=np.float32)
    b2 = np.asarray(b2, dtype=np.float32)
    gate_w = np.ascontiguousarray(gate_w, dtype=np.float32)
    gate_b = np.asarray(gate_b, dtype=np.float32)

    B, D = x.shape
    E, _, H = W1.shape
    O = W2.shape[2]
    assert E == N_CORES and B % N_CORES == 0 and D % P == 0
    H2 = H // 2
    assert H2 % P == 0 and O % 512 == 0
    core_ids = list(range(N_CORES))

    # ---- Host: gating + top-2 routing (the sharding decision) ----
    lg = x.astype(np.float64) @ gate_w.astype(np.float64) + gate_b.astype(np.float64)
    lg -= lg.max(axis=1, keepdims=True)
    probs = np.exp(lg)
    probs /= probs.sum(axis=1, keepdims=True)
    order = np.argsort(-probs, axis=1, kind="stable")[:, :2]
    p_top = np.take_along_axis(probs, order, axis=1)
    w_top = (p_top / p_top.sum(axis=1, keepdims=True)).astype(np.float32)  # [B, 2]

    idx_e, wt_e = [], []
    for e in range(E):
        m0 = order[:, 0] == e
        m1 = order[:, 1] == e
        sel = m0 | m1
        idx = np.nonzero(sel)[0]
        w = np.where(m0[sel], w_top[sel, 0], w_top[sel, 1]).astype(np.float32)
        idx_e.append(idx)
        wt_e.append(w)

    # pair heavy-with-light by routed token count for near-equal pair sums
    srt = sorted(range(E), key=lambda e: -len(idx_e[e]))
    pairs = [(srt[i], srt[E - 1 - i]) for i in range(E // 2)]
    Ca = max(CB, ((max(len(idx_e[a]) for a, _ in pairs) + P - 1) // P) * P)
    Cb = max(CB, ((max(len(idx_e[b]) for _, b in pairs) + P - 1) // P) * P)
    C = Ca + Cb

    # ---- Device: paired-expert half-H FFN ----
    key = ("pair", D, H2, O, Ca, Cb)
    if key not in _build_cache:
        _build_cache[key] = _build_pair(D, H2, O, Ca, Cb)
    nc_exp = _build_cache[key]

    blocks = _block_list(0, Ca) + _block_list(Ca, C)
    HO = H2 // P
    chunks = [_w1_chunks(HO, True), _w1_chunks(HO, False)]
    in_maps = []
    for i, (a, b) in enumerate(pairs):
        n_a, n_b = len(idx_e[a]), len(idx_e[b])
        xT_pad = np.zeros((D, C), dtype=_bf16_np)
        xT_pad[:, :n_a] = x[idx_e[a]].T.astype(_bf16_np)
        xT_pad[:, Ca:Ca + n_b] = x[idx_e[b]].T.astype(_bf16_np)
        x_packed = _pack_x(xT_pad, blocks, D, C)
        wt_pad = np.zeros(C, dtype=np.float32)
        wt_pad[:n_a] = wt_e[a]
        wt_pad[Ca:Ca + n_b] = wt_e[b]
        wt_m = np.ascontiguousarray(wt_pad.reshape(C // P, P).T)
        for half in range(2):
            sl = slice(half * H2, (half + 1) * H2)
            in_maps.append({
                "xT": x_packed,
                "w10": _pack_w1(W1[a][:, sl].astype(_bf16_np), chunks[0], D, H2),
                "w11": _pack_w1(W1[b][:, sl].astype(_bf16_np), chunks[1], D, H2),
                "w20": _pack_w2(W2[a][sl, :].astype(_bf16_np), H2, O, 2),
                "w21": _pack_w2(W2[b][sl, :].astype(_bf16_np), H2, O, 2),
                "b10": np.ascontiguousarray(b1[a][sl].reshape(H2 // P, P).T),
                "b11": np.ascontiguousarray(b1[b][sl].reshape(H2 // P, P).T),
                "wt": wt_m,
            })
    res = run_bass_kernel_spmd(nc_exp, in_maps, core_ids=core_ids)

    # ---- Host: sum the two half-H contributions, un-permute, combine ----
    out = np.zeros((B, O), dtype=np.float32)
    for i, (a, b) in enumerate(pairs):
        n_a, n_b = len(idx_e[a]), len(idx_e[b])
        yp = res.results[2 * i]["y"] + res.results[2 * i + 1]["y"]
        if n_a:
            out[idx_e[a]] += yp[:n_a]
        if n_b:
            out[idx_e[b]] += yp[Ca:Ca + n_b]
    if np.any(b2):
        out += w_top[:, 0, None] * b2[order[:, 0]] + w_top[:, 1, None] * b2[order[:, 1]]
    return out
